# revision 3
# baseline (speedup 1.0000x reference)
"""AttributeGNN Trainium2 kernel.

Problem: B=16384, A=8, E=512, fp32.
  src  = broadcast(image_features)                      [B,A,E]
  x    = concat([src, edge_attributes], -1)             [B,A,2E]
  agg  = x @ linear_w.T + linear_b                      [B,A,E]
  attr = einsum('bai,aij->baj', agg, fwd_proj)          [B,A,E]   (output 0)
  proj = einsum('bai,aij->baj', attr, bwd_proj)         [B,A,E]
  ind  = sum_a relu(proj) * self_weighted[0,a,0]        [B,E]     (output 1)

Strategy: data-parallel over B across 8 NeuronCores (2048 rows/core).
On device (per core), feature-major intermediates with fp32r matmuls:
  h        = img_fm.T-chunks @ lwT (+bias)              (phase A, once)
  agg_fm   = lwT[edge-part].T @ edge_fm + h             (per a, per b-tile)
  attr_bm  = agg-block.T @ fwd[a]                       -> attr output (batch-major)
  proj_bm  = agg-block.T @ FB[a],  FB[a]=fwd[a]@bwd[a]  (host-precomputed)
  ind      = sum_a w_a * relu(proj_bm)
Batch-major -> feature-major input transposes are done on the PE with an
identity matrix. linear_w.T and FB are precomputed on host (tiny).
"""

import numpy as np

import concourse.bacc as bacc
import concourse.mybir as mybir
import concourse.tile as tile
from concourse.bass_utils import run_bass_kernel_spmd
from concourse.masks import make_identity

P = 128
N_CORES = 8
B = 16384
A = 8
E = 512
BC = B // N_CORES          # 2048 batch rows per core
EC = E // P                # 4 feature chunks
BT = 512                   # batch tile (matmul moving dim)
NBT = BC // BT             # 4 batch tiles
NBB = BT // P              # 4 partition blocks per batch tile

F32 = mybir.dt.float32
F32R = mybir.dt.float32r


def build_kernel(w_vals):
    """Build + compile the per-core Bass module. w_vals: tuple of A python
    floats (self_weighted values), specialized into the instruction stream."""
    nc = bacc.Bacc("TRN2", target_bir_lowering=False, debug=False)

    img = nc.dram_tensor("img", [BC, E], F32R, kind="ExternalInput")
    edge = nc.dram_tensor("edge", [BC, A, E], F32R, kind="ExternalInput")
    lwt = nc.dram_tensor("lwt", [2 * E, E], F32R, kind="ExternalInput")   # linear_w.T
    lb = nc.dram_tensor("lb", [E], F32, kind="ExternalInput")
    fwd = nc.dram_tensor("fwd", [A, E, E], F32R, kind="ExternalInput")
    fb = nc.dram_tensor("fb", [A, E, E], F32R, kind="ExternalInput")      # fwd@bwd
    attr = nc.dram_tensor("attr", [BC, A, E], F32, kind="ExternalOutput")
    indiv = nc.dram_tensor("indiv", [BC, E], F32, kind="ExternalOutput")

    img_ap = img.ap()
    edge_ap = edge.ap()
    attr_ap = attr.ap()
    indiv_ap = indiv.ap()

    with tile.TileContext(nc) as tc:
        with (
            tc.tile_pool(name="const", bufs=1) as const_pool,
            tc.tile_pool(name="wstream", bufs=2) as w_pool,
            tc.tile_pool(name="stream", bufs=2) as s_pool,
            tc.tile_pool(name="tmp", bufs=2) as t_pool,
            tc.tile_pool(name="psum", bufs=7, space="PSUM") as ps_pool,
        ):
            # --- constants ---
            ident_f32 = const_pool.tile([P, P], F32)
            make_identity(nc, ident_f32[:])
            ident = const_pool.tile([P, P], F32R)
            nc.vector.tensor_copy(ident[:], ident_f32[:])
            lwt_sb = const_pool.tile([P, 2 * EC, E], F32R)   # [k=128, kc=8, i=512]
            nc.sync.dma_start(lwt_sb[:], lwt.ap().rearrange("(kc p) i -> p kc i", p=P))
            lb_sb = const_pool.tile([P, EC], F32)
            nc.sync.dma_start(lb_sb[:], lb.ap().rearrange("(ic p) -> p ic", p=P))
            h_sb = const_pool.tile([P, EC, BC], F32R)        # [i=128, ic, b]
            acc = const_pool.tile([P, NBT * NBB, E], F32)    # [b=128, bt*4+bb, j]

            def transpose_in(bm_tile, fm_tile):
                # bm_tile: [p=b(128), bb(4), k(512)] -> fm_tile: [p=k(128), kc(4), b(512)]
                for kc in range(EC):
                    ps = ps_pool.tile([P, BT], F32R, tag="ps")
                    for bb in range(NBB):
                        nc.tensor.transpose(
                            ps[:, bb * P:(bb + 1) * P],
                            bm_tile[:, bb, kc * P:(kc + 1) * P],
                            ident[:],
                        )
                    nc.vector.tensor_copy(fm_tile[:, kc, :], ps[:])

            # --- phase A: h = img-part matmul + bias ---
            for bt in range(NBT):
                img_bm = s_pool.tile([P, NBB, E], F32R, tag="x_bm")
                nc.sync.dma_start(
                    img_bm[:],
                    img_ap[bt * BT:(bt + 1) * BT, :].rearrange("(bb p) k -> p bb k", p=P),
                )
                img_fm = s_pool.tile([P, EC, BT], F32R, tag="x_fm")
                transpose_in(img_bm, img_fm)
                for ic in range(EC):
                    ps = ps_pool.tile([P, BT], F32, tag="ps")
                    for kc in range(EC):
                        nc.tensor.matmul(
                            ps[:],
                            lwt_sb[:, kc, ic * P:(ic + 1) * P],
                            img_fm[:, kc, :],
                            start=(kc == 0),
                            stop=(kc == EC - 1),
                        )
                    nc.vector.tensor_scalar_add(
                        h_sb[:, ic, bt * BT:(bt + 1) * BT], ps[:], lb_sb[:, ic:ic + 1]
                    )

            # --- phase B: per (a, b-tile) ---
            for a in range(A):
                fwd_sb = w_pool.tile([P, EC, E], F32R, tag="fwd")
                nc.sync.dma_start(
                    fwd_sb[:], fwd.ap()[a].rearrange("(ic p) j -> p ic j", p=P)
                )
                fb_sb = w_pool.tile([P, EC, E], F32R, tag="fb")
                nc.sync.dma_start(
                    fb_sb[:], fb.ap()[a].rearrange("(ic p) j -> p ic j", p=P)
                )
                for bt in range(NBT):
                    bts = slice(bt * BT, (bt + 1) * BT)
                    edge_bm = s_pool.tile([P, NBB, E], F32R, tag="x_bm")
                    nc.sync.dma_start(
                        edge_bm[:],
                        edge_ap[bts, a, :].rearrange("(bb p) k -> p bb k", p=P),
                    )
                    edge_fm = s_pool.tile([P, EC, BT], F32R, tag="x_fm")
                    transpose_in(edge_bm, edge_fm)

                    # S1: agg = lwT[edge].T @ edge_fm + h
                    agg = s_pool.tile([P, EC, BT], F32R, tag="agg")
                    for ic in range(EC):
                        ps = ps_pool.tile([P, BT], F32, tag="ps")
                        for kc in range(EC):
                            nc.tensor.matmul(
                                ps[:],
                                lwt_sb[:, EC + kc, ic * P:(ic + 1) * P],
                                edge_fm[:, kc, :],
                                start=(kc == 0),
                                stop=(kc == EC - 1),
                            )
                        nc.vector.tensor_tensor(
                            agg[:, ic, :], ps[:], h_sb[:, ic, bts], mybir.AluOpType.add
                        )

                    # S2: attr (batch-major) = agg-block.T @ fwd[a]
                    attr_out = s_pool.tile([P, NBB, E], F32, tag="attr_out")
                    for bb in range(NBB):
                        ps = ps_pool.tile([P, E], F32, tag="ps")
                        for ic in range(EC):
                            nc.tensor.matmul(
                                ps[:],
                                agg[:, ic, bb * P:(bb + 1) * P],
                                fwd_sb[:, ic, :],
                                start=(ic == 0),
                                stop=(ic == EC - 1),
                            )
                        nc.scalar.copy(attr_out[:, bb, :], ps[:])
                    nc.sync.dma_start(
                        attr_ap[bts, a, :].rearrange("(bb p) j -> p bb j", p=P),
                        attr_out[:],
                    )

                    # S3: proj (batch-major) = agg-block.T @ FB[a]; relu; accumulate
                    for bb in range(NBB):
                        ps = ps_pool.tile([P, E], F32, tag="ps")
                        for ic in range(EC):
                            nc.tensor.matmul(
                                ps[:],
                                agg[:, ic, bb * P:(bb + 1) * P],
                                fb_sb[:, ic, :],
                                start=(ic == 0),
                                stop=(ic == EC - 1),
                            )
                        idx = bt * NBB + bb
                        w_a = float(w_vals[a])
                        if a == 0:
                            nc.scalar.activation(
                                acc[:, idx, :], ps[:], mybir.ActivationFunctionType.Relu
                            )
                            if w_a != 1.0:
                                nc.vector.tensor_scalar_mul(
                                    acc[:, idx, :], acc[:, idx, :], w_a
                                )
                        else:
                            tmp = t_pool.tile([P, E], F32, tag="relu")
                            nc.scalar.activation(
                                tmp[:], ps[:], mybir.ActivationFunctionType.Relu
                            )
                            if w_a != 1.0:
                                nc.vector.tensor_scalar_mul(tmp[:], tmp[:], w_a)
                            nc.vector.tensor_tensor(
                                acc[:, idx, :], acc[:, idx, :], tmp[:],
                                mybir.AluOpType.add,
                            )

            # --- phase C: write individuals ---
            for bt in range(NBT):
                nc.sync.dma_start(
                    indiv_ap[bt * BT:(bt + 1) * BT, :].rearrange(
                        "(bb p) j -> p bb j", p=P
                    ),
                    acc[:, bt * NBB:(bt + 1) * NBB, :],
                )

    nc.compile()
    return nc


_CACHE = {}


def _get_kernel(w_vals):
    key = tuple(round(float(v), 12) for v in w_vals)
    if key not in _CACHE:
        _CACHE[key] = build_kernel(key)
    return _CACHE[key]


def prepare_in_maps(inputs):
    img = np.ascontiguousarray(np.asarray(inputs["image_features"], dtype=np.float32))
    edge = np.ascontiguousarray(np.asarray(inputs["edge_attributes"], dtype=np.float32))
    lw = np.asarray(inputs["linear_w"], dtype=np.float32)
    lb = np.ascontiguousarray(np.asarray(inputs["linear_b"], dtype=np.float32))
    fwd = np.ascontiguousarray(np.asarray(inputs["fwd_proj"], dtype=np.float32))
    bwd = np.asarray(inputs["bwd_proj"], dtype=np.float32)
    sw = np.asarray(inputs["self_weighted"], dtype=np.float32).reshape(A)

    lwt = np.ascontiguousarray(lw.T)                                   # [2E, E]
    fb = np.einsum(
        "aij,ajk->aik", fwd.astype(np.float64), bwd.astype(np.float64)
    ).astype(np.float32)                                               # [A, E, E]

    in_maps = []
    for c in range(N_CORES):
        bs = slice(c * BC, (c + 1) * BC)
        in_maps.append({
            "img": img[bs],
            "edge": edge[bs],
            "lwt": lwt,
            "lb": lb,
            "fwd": fwd,
            "fb": fb,
        })
    return in_maps, sw


def kernel(**inputs):
    in_maps, sw = prepare_in_maps(inputs)
    nc = _get_kernel(tuple(sw.tolist()))
    res = run_bass_kernel_spmd(nc, in_maps, core_ids=list(range(N_CORES)))
    attr = np.concatenate([res.results[c]["attr"] for c in range(N_CORES)], axis=0)
    indiv = np.concatenate([res.results[c]["indiv"] for c in range(N_CORES)], axis=0)
    return attr, indiv
